# revision 2
# baseline (speedup 1.0000x reference)
"""Trainium2 Bass kernel for nn_MemoryModule (channel-attention memory module).

Reference computation (per batch element b):
    k = key_w @ x + key_b          # [C, N] (1x1 conv over pixels)
    v = value_w @ x + value_b      # [C, N]
    E = k @ q^T                    # [C, C], q = proj_query.reshape(C, N)
    A = softmax(E, axis=-1)
    out = A^T @ v                  # [C, N]
    y = gamma * out + x

Sharding: data-parallel over batch B=8 across the 8 NeuronCores (one batch
element per core); the small CxC weights and the shared query are replicated.

Per-core kernel strategy (C=512, N=4096, fp32 inputs):
  - All matmuls run in float32r (TF32) mode: full PE rate at free-dim 512
    (vs 4x slower for plain fp32), with fp32 PSUM accumulation.
  - Phase 1: stream over 32 column-chunks of 128 pixels: compute
    kT[n,128 x C] = (x chunk)^T @ key_w^T + key_b, then immediately
    accumulate E[i_tile, :] += kT_chunk[:, i_tile]^T @ qT_chunk into 4
    persistent PSUM banks.  x is resident in SBUF; qT streams from HBM.
  - Softmax over the free dim of the 4 E PSUM tiles (max on DVE, exp with
    per-row bias + accumulated row-sum on the scalar engine).  A is kept
    unnormalized; 1/rowsum is folded into v's rows.
  - Phase 2: value conv v = value_w @ x + value_b, evicted with the fused
    (psum + vb) * (1/s) DVE op.  Runs on the PE while softmax finishes on
    DVE/ACT, keeping the PE warm.
  - Phase 3: out = A^T @ v per (row-tile, col-chunk); epilogue
    gamma*psum (ACT) + x (DVE), DMA to HBM.
"""

import numpy as np

import concourse.bass as bass
import concourse.tile as tile
from concourse import bacc, mybir
from concourse.bass_utils import run_bass_kernel_spmd

F32 = mybir.dt.float32
F32R = mybir.dt.float32r
AX = mybir.AxisListType
AF = mybir.ActivationFunctionType
ALU = mybir.AluOpType

C = 512
N = 4096
P = 128
CT = C // P          # 4 tiles over channel dim
NS = N // P          # 32 column sub-chunks of 128
NCH = N // 512       # 8 column chunks of 512
NCORES = 8

_cached = None


def _build_program():
    nc = bacc.Bacc("TRN2", target_bir_lowering=False, debug=False,
                   num_devices=NCORES)

    x_d = nc.dram_tensor("x", [C, N], F32, kind="ExternalInput").ap()
    qT_d = nc.dram_tensor("qT", [N, C], F32, kind="ExternalInput").ap()
    kwT_d = nc.dram_tensor("kwT", [C, C], F32, kind="ExternalInput").ap()
    vwT_d = nc.dram_tensor("vwT", [C, C], F32, kind="ExternalInput").ap()
    kb_d = nc.dram_tensor("kb", [P, C], F32, kind="ExternalInput").ap()
    vb_d = nc.dram_tensor("vb", [P, CT], F32, kind="ExternalInput").ap()
    gam_d = nc.dram_tensor("gam", [P, 1], F32, kind="ExternalInput").ap()
    out_d = nc.dram_tensor("out", [C, N], F32, kind="ExternalOutput").ap()

    with tile.TileContext(nc) as tc:
        with (
            tc.tile_pool(name="big", bufs=1) as big,
            tc.tile_pool(name="qp", bufs=3) as qp,
            tc.tile_pool(name="ktp", bufs=3) as ktp,
            tc.tile_pool(name="stp", bufs=4) as stp,
            tc.tile_pool(name="small", bufs=1) as small,
            tc.tile_pool(name="eps", bufs=1, space="PSUM") as eps,
            tc.tile_pool(name="wps", bufs=4, space="PSUM") as wps,
        ):
            # ---- resident loads ----
            x_sb = []
            kwT_sb = []
            vwT_sb = []
            v_sb = []
            for c in range(CT):
                xt = big.tile([P, N], F32R, tag=f"x{c}")
                nc.sync.dma_start(xt, x_d[c * P:(c + 1) * P, :].bitcast(F32R))
                x_sb.append(xt)
                kt = big.tile([P, C], F32R, tag=f"kw{c}")
                nc.sync.dma_start(kt, kwT_d[c * P:(c + 1) * P, :].bitcast(F32R))
                kwT_sb.append(kt)
                vt = big.tile([P, C], F32R, tag=f"vw{c}")
                nc.sync.dma_start(vt, vwT_d[c * P:(c + 1) * P, :].bitcast(F32R))
                vwT_sb.append(vt)
                v_sb.append(big.tile([P, N], F32R, tag=f"v{c}", name=f"v{c}"))
            kb_sb = small.tile([P, C], F32, tag="kb")
            nc.sync.dma_start(kb_sb, kb_d[:])
            vb_sb = small.tile([P, CT], F32, tag="vb")
            nc.sync.dma_start(vb_sb, vb_d[:])
            gam_sb = small.tile([P, 1], F32, tag="gam")
            nc.sync.dma_start(gam_sb, gam_d[:])

            e_ps = [eps.tile([P, 512], F32, tag=f"e{i}", name=f"e{i}")
                    for i in range(CT)]

            # ---- phase 1: kT conv + energy accumulation ----
            for ns in range(NS):
                qt = qp.tile([P, C], F32R, tag="qt")
                nc.sync.dma_start(qt, qT_d[ns * P:(ns + 1) * P, :].bitcast(F32R))
                kt_ps = wps.tile([P, 512], F32, tag="w")
                for c in range(CT):
                    nc.tensor.matmul(kt_ps[:], x_sb[c][:, ns * P:(ns + 1) * P],
                                     kwT_sb[c][:], start=(c == 0), stop=(c == CT - 1))
                kt = ktp.tile([P, C], F32R, tag="kt")
                nc.vector.tensor_add(kt[:], kt_ps[:], kb_sb[:])
                for i in range(CT):
                    nc.tensor.matmul(e_ps[i][:], kt[:, i * P:(i + 1) * P], qt[:],
                                     start=(ns == 0), stop=(ns == NS - 1))

            # ---- softmax over free dim of E tiles ----
            a_sb = []
            rs_sb = []
            for i in range(CT):
                nmx = small.tile([P, 1], F32, tag=f"nmx{i}")
                nc.vector.reduce_max(nmx[:], e_ps[i][:], axis=AX.X, negate=True)
                at = big.tile([P, 512], F32R, tag=f"a{i}")
                ssum = small.tile([P, 1], F32, tag=f"ssum{i}")
                nc.scalar.activation(at[:], e_ps[i][:], AF.Exp, bias=nmx[:, 0:1],
                                     scale=1.0, accum_out=ssum[:, 0:1])
                rs = small.tile([P, 1], F32, tag=f"rs{i}")
                nc.vector.reciprocal(rs[:], ssum[:])
                a_sb.append(at)
                rs_sb.append(rs)

            # ---- phase 2: value conv (keeps PE busy during softmax) ----
            for o in range(CT):
                for ch in range(NCH):
                    v_ps = wps.tile([P, 512], F32, tag="w")
                    for c in range(CT):
                        nc.tensor.matmul(v_ps[:], vwT_sb[c][:, o * P:(o + 1) * P],
                                         x_sb[c][:, ch * 512:(ch + 1) * 512],
                                         start=(c == 0), stop=(c == CT - 1))
                    # v' = (v_raw + vb) * (1/s)  [row-normalization folded in]
                    nc.vector.tensor_scalar(
                        out=v_sb[o][:, ch * 512:(ch + 1) * 512], in0=v_ps[:],
                        scalar1=vb_sb[:, o:o + 1], scalar2=rs_sb[o][:, 0:1],
                        op0=ALU.add, op1=ALU.mult)

            # ---- phase 3: out = A^T @ v' ; y = gamma*out + x ----
            for j in range(CT):
                for ch in range(NCH):
                    o_ps = wps.tile([P, 512], F32, tag="w")
                    for i in range(CT):
                        nc.tensor.matmul(o_ps[:], a_sb[i][:, j * P:(j + 1) * P],
                                         v_sb[i][:, ch * 512:(ch + 1) * 512],
                                         start=(i == 0), stop=(i == CT - 1))
                    st = stp.tile([P, 512], F32, tag="st")
                    nc.scalar.activation(st[:], o_ps[:], AF.Copy, bias=0.0,
                                         scale=gam_sb[:, 0:1])
                    ot = stp.tile([P, 512], F32, tag="ot")
                    nc.vector.tensor_add(
                        ot[:], st[:],
                        x_sb[j][:, ch * 512:(ch + 1) * 512].bitcast(F32))
                    nc.sync.dma_start(
                        out_d[j * P:(j + 1) * P, ch * 512:(ch + 1) * 512], ot[:])

    nc.compile()
    return nc


def _get_program():
    global _cached
    if _cached is None:
        _cached = _build_program()
    return _cached


def kernel(x, proj_query, key_w, key_b, value_w, value_b, gamma, **_unused):
    B, Cx, W, H = x.shape
    assert (B, Cx, W * H) == (NCORES, C, N)
    nc = _get_program()

    xb = np.ascontiguousarray(x.reshape(B, C, N), dtype=np.float32)
    qT = np.ascontiguousarray(proj_query.reshape(C, N).T, dtype=np.float32)
    kwT = np.ascontiguousarray(key_w.T, dtype=np.float32)
    vwT = np.ascontiguousarray(value_w.T, dtype=np.float32)
    kb = np.ascontiguousarray(np.broadcast_to(key_b, (P, C)), dtype=np.float32)
    vb = np.ascontiguousarray(value_b.reshape(CT, P).T, dtype=np.float32)
    gam = np.ascontiguousarray(np.broadcast_to(gamma.reshape(1, 1), (P, 1)),
                               dtype=np.float32)

    in_maps = [
        {"x": xb[b], "qT": qT, "kwT": kwT, "vwT": vwT, "kb": kb, "vb": vb,
         "gam": gam}
        for b in range(B)
    ]
    res = run_bass_kernel_spmd(nc, in_maps, list(range(NCORES)))
    out = np.stack([res.results[b]["out"] for b in range(B)])
    return out.reshape(B, C, W, H).astype(np.float32)


# revision 7
# speedup vs baseline: 160.6696x; 160.6696x over previous
"""Trainium2 Bass kernel for nn_MemoryModule (channel-attention memory module).

Reference computation (per batch element b):
    k = key_w @ x + key_b          # [C, N] (1x1 conv over pixels)
    v = value_w @ x + value_b      # [C, N]
    E = k @ q^T                    # [C, C], q = proj_query.reshape(C, N)
    A = softmax(E, axis=-1)
    out = A^T @ v                  # [C, N]
    y = gamma * out + x

Sharding: data-parallel over batch B=8 across the 8 NeuronCores (one batch
element per core); the small CxC weights and the shared query are replicated.

Per-core kernel strategy (C=512, N=4096, fp32 inputs):
  - All matmuls run in float32r (TF32) mode: full PE rate at free-dim 512
    (vs 4x slower for plain fp32), with fp32 PSUM accumulation.
    Uniform shape: stationary [128,128], moving [128,512], 512 matmuls total.
  - Phase 1 streams over 32 column-chunks of 128 pixels: kT_chunk =
    (x chunk)^T @ key_w^T + key_b (PSUM -> SBUF with bias add on DVE), then
    E[i_tile,:] += kT_chunk[:,i_tile]^T @ qT_chunk accumulates into 4
    persistent PSUM banks.  x streams into resident SBUF tiles in 512-col
    pieces just ahead of use; qT streams with a 4-deep prefetch.  Half the
    value-conv chunks are interleaved (every other chunk from ns=5) --
    v-conv reads only SBUF-resident x, so it fills the PE while the DMA
    stream catches up on the qT/x deficit.
  - Softmax over the free dim of the E PSUM tiles (negated max on DVE, exp
    with per-row bias + accumulated row-sum on ACT, reciprocal on DVE);
    A is then scaled by 1/rowsum in place.  Runs while v-conv matmuls
    continue on the PE (no PE idle).
  - Remaining v-conv chunks, then out = A^T @ v per (row-tile, col-chunk);
    epilogue gamma*psum (ACT) + x (DVE), DMA out.
"""

import numpy as np

import concourse.bass as bass
import concourse.tile as tile
from concourse import bacc, mybir
from concourse.bass_utils import run_bass_kernel_spmd

F32 = mybir.dt.float32
F32R = mybir.dt.float32r
AX = mybir.AxisListType
AF = mybir.ActivationFunctionType
ALU = mybir.AluOpType

C = 512
N = 4096
P = 128
CT = C // P          # 4 tiles over channel dim
NS = N // P          # 32 column sub-chunks of 128
NCH = N // 512       # 8 column chunks of 512
NCORES = 8

_cached = {}


def _build_program(repeat=1):
    nc = bacc.Bacc("TRN2", target_bir_lowering=False, debug=False,
                   num_devices=NCORES)

    x_d = nc.dram_tensor("x", [C, N], F32, kind="ExternalInput").ap()
    qT_d = nc.dram_tensor("qT", [N, C], F32, kind="ExternalInput").ap()
    kwT_d = nc.dram_tensor("kwT", [C, C], F32, kind="ExternalInput").ap()
    vwT_d = nc.dram_tensor("vwT", [C, C], F32, kind="ExternalInput").ap()
    kb_d = nc.dram_tensor("kb", [P, C], F32, kind="ExternalInput").ap()
    vb_d = nc.dram_tensor("vb", [P, CT], F32, kind="ExternalInput").ap()
    gam_d = nc.dram_tensor("gam", [P, 1], F32, kind="ExternalInput").ap()
    out_d = nc.dram_tensor("out", [C, N], F32, kind="ExternalOutput").ap()

    # v-conv chunk schedule: ch-major so the chunks interleaved into phase 1
    # only read x pieces whose streaming DMA was already emitted (chunk vi
    # at ns=V_START+2*vi touches piece ch=vi//4, emitted at ns=4*(ch-1)).
    v_chunks = [(o, ch) for ch in range(NCH) for o in range(CT)]
    V_START = 7
    ph1_ns = list(range(V_START, NS, 2))

    from contextlib import ExitStack

    with tile.TileContext(nc) as tc:
        with (
            tc.tile_pool(name="big", bufs=1) as big,
            tc.tile_pool(name="qp", bufs=4) as qp,
            tc.tile_pool(name="ktp", bufs=6) as ktp,
            tc.tile_pool(name="stp", bufs=6) as stp,
            tc.tile_pool(name="small", bufs=1) as small,
            tc.tile_pool(name="wps", bufs=4, space="PSUM") as wps,
        ):
            for _rep in range(repeat):
                eps_stack = ExitStack()
                eps = eps_stack.enter_context(
                    tc.tile_pool(name="eps", bufs=1, space="PSUM"))
                # ---- ramp: first x piece + weights, interleaved ----
                x_sb = [big.tile([P, N], F32R, tag=f"x{c}", name=f"x{c}")
                        for c in range(CT)]
                kwT_sb = [big.tile([P, C], F32R, tag=f"kw{c}", name=f"kw{c}")
                          for c in range(CT)]
                vwT_sb = [big.tile([P, C], F32R, tag=f"vw{c}", name=f"vw{c}")
                          for c in range(CT)]
                v_sb = [big.tile([P, N], F32R, tag=f"v{c}", name=f"v{c}")
                        for c in range(CT)]
                for c in range(CT):
                    nc.sync.dma_start(x_sb[c][:, 0:512],
                                      x_d[c * P:(c + 1) * P, 0:512].bitcast(F32R))
                    nc.sync.dma_start(kwT_sb[c],
                                      kwT_d[c * P:(c + 1) * P, :].bitcast(F32R))
                kb_sb = small.tile([P, C], F32, tag="kb")
                nc.sync.dma_start(kb_sb, kb_d[:])
                # qt0 + x piece 1 + qt1-2 ahead of the value weights: the
                # energy stream and chunk-4 kT matmuls need them sooner.
                qt_pre = []
                for pre in range(3):
                    qt = qp.tile([P, C], F32R, tag="qt", name="qt")
                    nc.sync.dma_start(
                        qt, qT_d[pre * P:(pre + 1) * P, :].bitcast(F32R))
                    qt_pre.append(qt)
                    if pre == 0:
                        for c in range(CT):
                            nc.sync.dma_start(
                                x_sb[c][:, 512:1024],
                                x_d[c * P:(c + 1) * P, 512:1024].bitcast(F32R))
                for c in range(CT):
                    nc.sync.dma_start(vwT_sb[c],
                                      vwT_d[c * P:(c + 1) * P, :].bitcast(F32R))
                vb_sb = small.tile([P, CT], F32, tag="vb")
                nc.sync.dma_start(vb_sb, vb_d[:])
                gam_sb = small.tile([P, 1], F32, tag="gam")
                nc.sync.dma_start(gam_sb, gam_d[:])

                e_ps = [eps.tile([P, 512], F32, tag=f"e{i}", name=f"e{i}")
                        for i in range(CT)]

                def emit_v_chunk(o, ch, pool=wps):
                    v_ps = pool.tile([P, 512], F32, tag="w", name="v_ps")
                    for c in range(CT):
                        nc.tensor.matmul(v_ps[:], vwT_sb[c][:, o * P:(o + 1) * P],
                                         x_sb[c][:, ch * 512:(ch + 1) * 512],
                                         start=(c == 0), stop=(c == CT - 1))
                    nc.vector.tensor_scalar_add(
                        v_sb[o][:, ch * 512:(ch + 1) * 512], v_ps[:],
                        vb_sb[:, o:o + 1])

                # ---- phase 1: kT conv + energy accumulation + v interleave ----
                vi = 0
                for ns in range(NS):
                    piece = ns // 4 + 1
                    if ns % 4 == 0 and 1 < piece < NCH:
                        for c in range(CT):
                            nc.sync.dma_start(
                                x_sb[c][:, piece * 512:(piece + 1) * 512],
                                x_d[c * P:(c + 1) * P,
                                    piece * 512:(piece + 1) * 512].bitcast(F32R))
                    if ns < 3:
                        qt = qt_pre[ns]
                    else:
                        qt = qp.tile([P, C], F32R, tag="qt", name="qt")
                        nc.sync.dma_start(
                            qt, qT_d[ns * P:(ns + 1) * P, :].bitcast(F32R))
                    kt_ps = wps.tile([P, 512], F32, tag="w", name="kt_ps")
                    for c in range(CT):
                        nc.tensor.matmul(kt_ps[:], x_sb[c][:, ns * P:(ns + 1) * P],
                                         kwT_sb[c][:], start=(c == 0),
                                         stop=(c == CT - 1))
                    kt = ktp.tile([P, C], F32R, tag="kt", name="kt")
                    nc.vector.tensor_add(kt[:], kt_ps[:], kb_sb[:])
                    for i in range(CT):
                        nc.tensor.matmul(e_ps[i][:], kt[:, i * P:(i + 1) * P], qt[:],
                                         start=(ns == 0), stop=(ns == NS - 1))
                    if ns in ph1_ns:
                        emit_v_chunk(*v_chunks[vi])
                        vi += 1

                # ---- softmax over free dim of E tiles; scale A by 1/rowsum ----
                a_sb = []
                for i in range(CT):
                    nmx = small.tile([P, 1], F32, tag=f"nmx{i}", name=f"nmx{i}")
                    nc.vector.reduce_max(nmx[:], e_ps[i][:], axis=AX.X, negate=True)
                    at = big.tile([P, 512], F32R, tag=f"a{i}", name=f"a{i}")
                    ssum = small.tile([P, 1], F32, tag=f"ssum{i}", name=f"ssum{i}")
                    nc.scalar.activation(at[:], e_ps[i][:], AF.Exp, bias=nmx[:, 0:1],
                                         scale=1.0, accum_out=ssum[:, 0:1])
                    rs = small.tile([P, 1], F32, tag=f"rs{i}", name=f"rs{i}")
                    nc.vector.reciprocal(rs[:], ssum[:])
                    nc.vector.tensor_scalar(
                        out=at[:], in0=at[:], scalar1=rs[:, 0:1],
                        scalar2=gam_sb[:, 0:1], op0=ALU.mult, op1=ALU.mult)
                    a_sb.append(at)

                eps_stack.close()
                wps2_stack = ExitStack()
                wps2 = wps2_stack.enter_context(
                    tc.tile_pool(name="wps2", bufs=4, space="PSUM"))
                pools = (wps, wps2)

                # ---- phase 2: remaining v-conv chunks ----
                for idx, (o, ch) in enumerate(v_chunks[vi:]):
                    emit_v_chunk(o, ch, pool=pools[idx % 2])

                # ---- phase 3: out = A^T @ v (gamma folded into A) ; +x ----
                for idx, (j, ch) in enumerate(
                        (j, ch) for j in range(CT) for ch in range(NCH)):
                    o_ps = pools[idx % 2].tile([P, 512], F32, tag="w",
                                               name="o_ps")
                    for i in range(CT):
                        nc.tensor.matmul(o_ps[:],
                                         a_sb[i][:, j * P:(j + 1) * P],
                                         v_sb[i][:, ch * 512:(ch + 1) * 512],
                                         start=(i == 0), stop=(i == CT - 1))
                    ot = stp.tile([P, 512], F32, tag="ot", name="ot")
                    nc.vector.tensor_add(
                        ot[:], o_ps[:],
                        x_sb[j][:, ch * 512:(ch + 1) * 512].bitcast(F32))
                    nc.sync.dma_start(
                        out_d[j * P:(j + 1) * P, ch * 512:(ch + 1) * 512],
                        ot[:])
                wps2_stack.close()

    nc.compile()
    return nc


def _get_program(repeat=1):
    if repeat not in _cached:
        _cached[repeat] = _build_program(repeat)
    return _cached[repeat]


def kernel(x, proj_query, key_w, key_b, value_w, value_b, gamma, **_unused):
    B, Cx, W, H = x.shape
    assert (B, Cx, W * H) == (NCORES, C, N)
    nc = _get_program()

    xb = np.ascontiguousarray(x.reshape(B, C, N), dtype=np.float32)
    qT = np.ascontiguousarray(proj_query.reshape(C, N).T, dtype=np.float32)
    kwT = np.ascontiguousarray(key_w.T, dtype=np.float32)
    vwT = np.ascontiguousarray(value_w.T, dtype=np.float32)
    kb = np.ascontiguousarray(np.broadcast_to(key_b, (P, C)), dtype=np.float32)
    vb = np.ascontiguousarray(value_b.reshape(CT, P).T, dtype=np.float32)
    gam = np.ascontiguousarray(np.broadcast_to(gamma.reshape(1, 1), (P, 1)),
                               dtype=np.float32)

    in_maps = [
        {"x": xb[b], "qT": qT, "kwT": kwT, "vwT": vwT, "kb": kb, "vb": vb,
         "gam": gam}
        for b in range(B)
    ]
    res = run_bass_kernel_spmd(nc, in_maps, list(range(NCORES)))
    out = np.stack([res.results[b]["out"] for b in range(B)])
    return out.reshape(B, C, W, H).astype(np.float32)


# revision 8
# speedup vs baseline: 200.5501x; 1.2482x over previous
"""G-trick variant: E = kw @ (x @ qT) + kb * S, S = colsum(qT).

Reassociating the energy computation through the [C,C] Gram matrix
G = x @ qT cuts the energy path from 256 matmuls (kT conv + E) to
128 (G) + 16 (kw@G) + 32 (S via ones-vector) + 4 (rank-1 bias) and
removes the 32 kT PSUM->SBUF evictions from the DVE.  Needs x in both
layouts (x and xT) -- the extra 8MB stream is the price.  All of the
v-conv is interleaved into phase 1.
"""

import numpy as np

import concourse.bass as bass
import concourse.tile as tile
from concourse import bacc, mybir
from concourse.bass_utils import run_bass_kernel_spmd

F32 = mybir.dt.float32
F32R = mybir.dt.float32r
AX = mybir.AxisListType
AF = mybir.ActivationFunctionType
ALU = mybir.AluOpType

C = 512
N = 4096
P = 128
CT = C // P
NS = N // P
NCH = N // 512
NCORES = 8

_cached = {}


def _build_program(repeat=1):
    from contextlib import ExitStack

    nc = bacc.Bacc("TRN2", target_bir_lowering=False, debug=False,
                   num_devices=NCORES)

    x_d = nc.dram_tensor("x", [C, N], F32, kind="ExternalInput").ap()
    xT_d = nc.dram_tensor("xT", [N, C], F32, kind="ExternalInput").ap()
    qT_d = nc.dram_tensor("qT", [N, C], F32, kind="ExternalInput").ap()
    kwT_d = nc.dram_tensor("kwT", [C, C], F32, kind="ExternalInput").ap()
    vwT_d = nc.dram_tensor("vwT", [C, C], F32, kind="ExternalInput").ap()
    kbr_d = nc.dram_tensor("kbr", [1, C], F32, kind="ExternalInput").ap()
    vb_d = nc.dram_tensor("vb", [P, CT], F32, kind="ExternalInput").ap()
    gam_d = nc.dram_tensor("gam", [P, 1], F32, kind="ExternalInput").ap()
    ones_d = nc.dram_tensor("ones", [P, 1], F32, kind="ExternalInput").ap()
    out_d = nc.dram_tensor("out", [C, N], F32, kind="ExternalOutput").ap()

    # all 32 v chunks interleave into phase 1, ch-major so x pieces are ready
    v_chunks = [(o, ch) for ch in range(NCH) for o in range(CT)]

    with tile.TileContext(nc) as tc:
        with (
            tc.tile_pool(name="big", bufs=1) as big,
            tc.tile_pool(name="qp", bufs=4) as qp,
            tc.tile_pool(name="xtp", bufs=4) as xtp,
            tc.tile_pool(name="stp", bufs=6) as stp,
            tc.tile_pool(name="small", bufs=1) as small,
            tc.tile_pool(name="wps", bufs=3, space="PSUM") as wps,
        ):
            for _rep in range(repeat):
                gps_stack = ExitStack()
                gps = gps_stack.enter_context(
                    tc.tile_pool(name="gps", bufs=1, space="PSUM"))

                x_sb = [big.tile([P, N], F32R, tag=f"x{c}", name=f"x{c}")
                        for c in range(CT)]
                kwT_sb = [big.tile([P, C], F32R, tag=f"kw{c}", name=f"kw{c}")
                          for c in range(CT)]
                vwT_sb = [big.tile([P, C], F32R, tag=f"vw{c}", name=f"vw{c}")
                          for c in range(CT)]
                v_sb = [big.tile([P, N], F32R, tag=f"v{c}", name=f"v{c}")
                        for c in range(CT)]
                G_sb = [big.tile([P, C], F32R, tag=f"g{c}", name=f"g{c}")
                        for c in range(CT)]

                ones_sb = small.tile([P, 1], F32R, tag="ones")
                nc.sync.dma_start(ones_sb, ones_d[:].bitcast(F32R))

                # ramp: first xT/qT chunks come first so the PE starts early
                qt_pre = []
                xt_pre = []
                for pre in range(2):
                    xt = xtp.tile([P, C], F32R, tag="xt", name="xt")
                    nc.sync.dma_start(
                        xt, xT_d[pre * P:(pre + 1) * P, :].bitcast(F32R))
                    xt_pre.append(xt)
                    qt = qp.tile([P, C], F32R, tag="qt", name="qt")
                    nc.sync.dma_start(
                        qt, qT_d[pre * P:(pre + 1) * P, :].bitcast(F32R))
                    qt_pre.append(qt)
                for c in range(CT):
                    nc.sync.dma_start(
                        vwT_sb[c], vwT_d[c * P:(c + 1) * P, :].bitcast(F32R))
                vb_sb = small.tile([P, CT], F32, tag="vb")
                nc.sync.dma_start(vb_sb, vb_d[:])
                for c in range(CT):
                    nc.sync.dma_start(
                        x_sb[c][:, 0:512],
                        x_d[c * P:(c + 1) * P, 0:512].bitcast(F32R))
                gam_sb = small.tile([P, 1], F32, tag="gam")
                nc.sync.dma_start(gam_sb, gam_d[:])

                g_ps = [gps.tile([P, 512], F32, tag=f"gp{i}", name=f"gp{i}")
                        for i in range(CT)]
                s_ps = gps.tile([P, 512], F32, tag="sp", name="s_ps")

                def emit_v_chunk(o, ch, pool):
                    v_ps = pool.tile([P, 512], F32, tag="w", name="v_ps")
                    for c in range(CT):
                        nc.tensor.matmul(v_ps[:], vwT_sb[c][:, o * P:(o + 1) * P],
                                         x_sb[c][:, ch * 512:(ch + 1) * 512],
                                         start=(c == 0), stop=(c == CT - 1))
                    nc.vector.tensor_scalar_add(
                        v_sb[o][:, ch * 512:(ch + 1) * 512], v_ps[:],
                        vb_sb[:, o:o + 1])

                # ---- phase 1: G/S accumulation + full v-conv interleave ----
                for ns in range(NS):
                    piece = ns // 4 + 1
                    if ns % 4 == 1 and piece < NCH:
                        for c in range(CT):
                            nc.sync.dma_start(
                                x_sb[c][:, piece * 512:(piece + 1) * 512],
                                x_d[c * P:(c + 1) * P,
                                    piece * 512:(piece + 1) * 512].bitcast(F32R))
                    if ns == 16:
                        for c in range(CT):
                            nc.sync.dma_start(
                                kwT_sb[c],
                                kwT_d[c * P:(c + 1) * P, :].bitcast(F32R))
                        kbr_sb = small.tile([1, C], F32R, tag="kbr",
                                            name="kbr")
                        nc.sync.dma_start(kbr_sb, kbr_d[:].bitcast(F32R))
                    if ns < 2:
                        xt, qt = xt_pre[ns], qt_pre[ns]
                    else:
                        xt = xtp.tile([P, C], F32R, tag="xt", name="xt")
                        nc.sync.dma_start(
                            xt, xT_d[ns * P:(ns + 1) * P, :].bitcast(F32R))
                        qt = qp.tile([P, C], F32R, tag="qt", name="qt")
                        nc.sync.dma_start(
                            qt, qT_d[ns * P:(ns + 1) * P, :].bitcast(F32R))
                    for ct in range(CT):
                        nc.tensor.matmul(g_ps[ct][:], xt[:, ct * P:(ct + 1) * P],
                                         qt[:], start=(ns == 0),
                                         stop=(ns == NS - 1))
                    nc.tensor.matmul(s_ps[0:1, :], ones_sb[:], qt[:],
                                     start=(ns == 0), stop=(ns == NS - 1))
                    emit_v_chunk(*v_chunks[ns], pool=wps)

                # ---- evict G and S to SBUF ----
                for ct in range(CT):
                    nc.vector.tensor_copy(G_sb[ct][:], g_ps[ct][:])
                s_sb = small.tile([1, C], F32R, tag="s_sb", name="s_sb")
                nc.vector.tensor_copy(s_sb[:], s_ps[0:1, :])

                gps_stack.close()
                eps_stack = ExitStack()
                eps = eps_stack.enter_context(
                    tc.tile_pool(name="eps", bufs=1, space="PSUM"))
                e_ps = [eps.tile([P, 512], F32, tag=f"e{i}", name=f"e{i}")
                        for i in range(CT)]

                # ---- E = kw @ G + kb . S ----
                for i in range(CT):
                    for ct in range(CT):
                        nc.tensor.matmul(e_ps[i][:],
                                         kwT_sb[ct][:, i * P:(i + 1) * P],
                                         G_sb[ct][:], start=(ct == 0),
                                         stop=False)
                    nc.tensor.matmul(e_ps[i][:],
                                     kbr_sb[0:1, i * P:(i + 1) * P],
                                     s_sb[0:1, :], start=False, stop=True)

                # ---- softmax; scale A by gamma/rowsum ----
                a_sb = []
                for i in range(CT):
                    nmx = small.tile([P, 1], F32, tag=f"nmx{i}", name=f"nmx{i}")
                    nc.vector.reduce_max(nmx[:], e_ps[i][:], axis=AX.X,
                                         negate=True)
                    at = big.tile([P, 512], F32R, tag=f"a{i}", name=f"a{i}")
                    ssum = small.tile([P, 1], F32, tag=f"ssum{i}",
                                      name=f"ssum{i}")
                    nc.scalar.activation(at[:], e_ps[i][:], AF.Exp,
                                         bias=nmx[:, 0:1], scale=1.0,
                                         accum_out=ssum[:, 0:1])
                    rs = small.tile([P, 1], F32, tag=f"rs{i}", name=f"rs{i}")
                    nc.vector.reciprocal(rs[:], ssum[:])
                    nc.vector.tensor_scalar(
                        out=at[:], in0=at[:], scalar1=rs[:, 0:1],
                        scalar2=gam_sb[:, 0:1], op0=ALU.mult, op1=ALU.mult)
                    a_sb.append(at)

                eps_stack.close()
                fps_stack = ExitStack()
                fps = fps_stack.enter_context(
                    tc.tile_pool(name="fps", bufs=4, space="PSUM"))
                pools = (wps, fps)

                # ---- final: out = A^T @ v (gamma folded into A) ; +x ----
                for idx, (j, ch) in enumerate(
                        (j, ch) for j in range(CT) for ch in range(NCH)):
                    o_ps = pools[idx % 2].tile([P, 512], F32, tag="w",
                                               name="o_ps")
                    for i in range(CT):
                        nc.tensor.matmul(o_ps[:],
                                         a_sb[i][:, j * P:(j + 1) * P],
                                         v_sb[i][:, ch * 512:(ch + 1) * 512],
                                         start=(i == 0), stop=(i == CT - 1))
                    ot = stp.tile([P, 512], F32, tag="ot", name="ot")
                    nc.vector.tensor_add(
                        ot[:], o_ps[:],
                        x_sb[j][:, ch * 512:(ch + 1) * 512].bitcast(F32))
                    nc.sync.dma_start(
                        out_d[j * P:(j + 1) * P, ch * 512:(ch + 1) * 512],
                        ot[:])
                fps_stack.close()

    nc.compile()
    return nc


def _get_program(repeat=1):
    if repeat not in _cached:
        _cached[repeat] = _build_program(repeat)
    return _cached[repeat]


def make_in_maps(x, proj_query, key_w, key_b, value_w, value_b, gamma):
    """Per-core input dicts: batch-parallel shards + replicated weights."""
    B = x.shape[0]
    xb = np.ascontiguousarray(x.reshape(B, C, N), dtype=np.float32)
    qT = np.ascontiguousarray(proj_query.reshape(C, N).T, dtype=np.float32)
    kwT = np.ascontiguousarray(key_w.T, dtype=np.float32)
    vwT = np.ascontiguousarray(value_w.T, dtype=np.float32)
    kbr = np.ascontiguousarray(key_b.reshape(1, C), dtype=np.float32)
    vb = np.ascontiguousarray(value_b.reshape(CT, P).T, dtype=np.float32)
    gam = np.ascontiguousarray(np.broadcast_to(np.asarray(gamma).reshape(1, 1),
                                               (P, 1)), dtype=np.float32)
    ones = np.ones((P, 1), np.float32)
    return [
        {"x": xb[b], "xT": np.ascontiguousarray(xb[b].T), "qT": qT,
         "kwT": kwT, "vwT": vwT, "kbr": kbr, "vb": vb, "gam": gam,
         "ones": ones}
        for b in range(B)
    ]


def kernel(x, proj_query, key_w, key_b, value_w, value_b, gamma, **_unused):
    B, Cx, W, H = x.shape
    assert (B, Cx, W * H) == (NCORES, C, N)
    nc = _get_program()
    in_maps = make_in_maps(x, proj_query, key_w, key_b, value_w, value_b,
                           gamma)
    res = run_bass_kernel_spmd(nc, in_maps, list(range(NCORES)))
    out = np.stack([res.results[b]["out"] for b in range(B)])
    return out.reshape(B, C, W, H).astype(np.float32)
